# revision 31
# baseline (speedup 1.0000x reference)
"""Trainium2 Bass kernel: 6-head causal self-attention (nn_MultiHead).

Strategy: pure data-parallel over batch B=256 across 8 NeuronCores
(32 batches/core, no collectives). Per batch, on-chip layout keeps the
contraction dim on SBUF partitions everywhere:

  host:    x [B,T,D] -> xT [B,D,T] fp16;  W_qkv -> Wq/Wk/Wv [D, H*HS] fp16;
           bias is applied host-side (zeros in practice)
  proj:    qT/kT [(h e), t] = W.T @ xT    (PE; two batches fused, N=512;
           q and k of one 128-row chunk share a 2-bank PSUM tile and one
           ACT copy)
  scores:  S^T [s, t] = kT_h^T @ qT_h per head (K=64), both heads of a
           pair in one 2-bank PSUM tile; causal skip: the s-chunk-1
           matmul only covers t >= 128 (N=128)
  softmax: P = exp(S/8), one paired ACT op [128,2,384] -> fp16 SBUF; the
           causal mask is applied POST-exp as a 0/1 multiply on the
           diagonal blocks (DVE, fp16, one strided op per stage)
  PV:      O_aug [128, t] = V_aug^T @ P^T where V_aug carries 64 extra
           all-ones columns, so rows 64:128 of the PSUM result hold the
           softmax denominators PRE-REPLICATED across 64 partitions (the
           extra matmul rows are free: PE cost depends only on N)
  norm:    fast reciprocal over the whole O_aug tile (DVE custom op
           reading PSUM; base partition must be 0), then
           O^T *= rr[64:128] while copying PSUM->SBUF (DVE) -- no
           partition broadcast or gather anywhere
  out:     y [t, d] = O^T.T @ W_out (PE), ACT copy PSUM->SBUF, one DMA
           per batch

A slot pipeline runs the six (sub-batch, head-pair) stages per
macro-batch: slot i emits PV/recip/normalize(i-3), scores+exp(i),
P-mask(i), out-projection(i-4); q/k projections are hoisted two slots
ahead of each macro-batch boundary and v projections split across its
first two slots, so the PE never drains and PSUM stays within the
8-bank budget (2x2-bank score/proj pairs + 4 single banks). Matmul
operands are fp16 (1 cycle/row streaming); accumulation is fp32 PSUM.
"""

import sys

import numpy as np

if "/opt/trn_rl_repo" not in sys.path:
    sys.path.insert(0, "/opt/trn_rl_repo")

from contextlib import ExitStack

import concourse.bass as bass  # noqa: F401
import concourse.tile as tile
from concourse import bacc, mybir
from concourse.bass_utils import run_bass_kernel_spmd

B, T, D, H, HS = 256, 256, 384, 6, 64
NCORES = 8
BPC = B // NCORES  # batches per core
F32 = mybir.dt.float32
F16 = mybir.dt.float16
EXP = mybir.ActivationFunctionType.Exp
SCALE = 1.0 / 8.0  # 1/sqrt(HS)


def _emit(ctx, tc, aps, bpc):
    nc = tc.nc
    xT, wq, wk, wv, wo, tri4, vone, y = aps
    assert bpc % 2 == 0
    nmb = bpc // 2  # macro-batches of 2

    singles = ctx.enter_context(tc.tile_pool(name="singles", bufs=1))
    xpool = ctx.enter_context(tc.tile_pool(name="xp", bufs=6))
    qkpool = ctx.enter_context(tc.tile_pool(name="qkp", bufs=6))
    ppool = ctx.enter_context(tc.tile_pool(name="pp", bufs=6))
    opool = ctx.enter_context(tc.tile_pool(name="op", bufs=12))
    rrpool = ctx.enter_context(tc.tile_pool(name="rrp", bufs=3))
    ypool = ctx.enter_context(tc.tile_pool(name="yp", bufs=4))
    ps_pair = ctx.enter_context(tc.tile_pool(name="ps_pair", bufs=2, space="PSUM"))
    ps_small = ctx.enter_context(tc.tile_pool(name="ps_small", bufs=4, space="PSUM"))

    # Constants / weights, loaded once.
    def _load(name, src, shape, dt=F16):
        t = singles.tile(shape, dt, tag=name, name=name)
        nc.sync.dma_start(out=t, in_=src)
        return t

    wq_sb = [_load(f"wq{i}", wq[i * 128 : (i + 1) * 128, :], [128, D]) for i in range(3)]
    wk_sb = [_load(f"wk{i}", wk[i * 128 : (i + 1) * 128, :], [128, D]) for i in range(3)]
    wv_sb = [_load(f"wv{i}", wv[i * 128 : (i + 1) * 128, :], [128, D]) for i in range(3)]
    wo_sb = [_load(f"wo{i}", wo[i * 128 : (i + 1) * 128, :], [128, D]) for i in range(3)]
    tri4_sb = _load("tri4", tri4, [128, 512])
    tri4v = tri4_sb.rearrange("p (a b c) -> p a b c", b=2, c=128)

    # Persistent v_aug tiles [macro-parity][sub-batch][s-tile]: ones columns
    # are DMA'd once and survive all batches (the per-batch copy writes only
    # cols 0:64 of each 65-wide head block).
    va_all = []
    for par in range(2):
        subs = []
        for sub in range(2):
            pair = []
            for st in range(2):
                t = singles.tile(
                    [128, H * 128], F16, tag=f"va{par}{sub}{st}", name=f"va{par}{sub}{st}"
                )
                nc.sync.dma_start(
                    out=t.rearrange("p (h c) -> p h c", c=128)[:, :, 64:128],
                    in_=vone.rearrange("p (h c) -> p h c", c=64),
                )
                pair.append(t)
            subs.append(pair)
        va_all.append(subs)

    # xT viewed so two consecutive batches concatenate along the free dim:
    # [mb, d, (sub t)] per 128-row d-chunk
    def x2_src(mb, kc):
        return xT[2 * mb : 2 * mb + 2, kc * 128 : (kc + 1) * 128, :].rearrange(
            "b d t -> d b t"
        )

    # Pipeline state, keyed by global stage index g = mb*6 + sub*3 + hp.
    x_d = {}  # mb -> [3 x-tiles]
    qk_d = {}  # (mb, mt) -> [128, 1024] f16 (q cols 0:512, k cols 512:1024)
    pair_d = {}  # g -> [128, 1024] f32 psum (scores, both heads)
    p_d = {}  # g -> [128, 768] f16 (exp output, both heads)
    o_d = {}  # g -> [128, 512] f32 psum (PV out; rows 64:128 = denominators)
    rr_d = {}  # g -> [128, 512] f32 (rows 64:128 = denominator reciprocals)
    oT_d = {}  # (mb, sub) -> [3 oT tiles]

    def stage(i):
        if 0 <= i < nmb * 6:
            return i // 6, (i % 6) // 3, i % 3  # mb, sub, hp
        return None

    def emit_xload(mb):
        if mb >= nmb:
            return
        tiles = []
        for kc in range(3):
            t = xpool.tile([128, 2 * T], F16, tag="x", name="x")
            nc.sync.dma_start(
                out=t.rearrange("p (b t) -> p b t", t=T), in_=x2_src(mb, kc)
            )
            tiles.append(t)
        x_d[mb] = tiles

    def emit_qkproj(mb, mt):
        x_sb = x_d[mb]
        ps = ps_pair.tile([128, 1024], F32, tag="pair", name="qk_ps")
        for kc in range(3):
            nc.tensor.matmul(
                ps[:, 0:512],
                wq_sb[kc][:, mt * 128 : (mt + 1) * 128],
                x_sb[kc],
                start=(kc == 0),
                stop=(kc == 2),
            )
        for kc in range(3):
            nc.tensor.matmul(
                ps[:, 512:1024],
                wk_sb[kc][:, mt * 128 : (mt + 1) * 128],
                x_sb[kc],
                start=(kc == 0),
                stop=(kc == 2),
            )
        sb = qkpool.tile([128, 1024], F16, tag="qk", name="qk_sb")
        nc.scalar.copy(sb, ps)
        qk_d[(mb, mt)] = sb

    def emit_vproj(mb, sub):
        x_sb = x_d[mb]
        va_mb = va_all[mb % 2]
        for st in range(2):
            ps = ps_small.tile([128, D], F32, tag="small", name="v_ps")
            for kc in range(3):
                nc.tensor.matmul(
                    ps,
                    x_sb[kc][:, sub * T + st * 128 : sub * T + (st + 1) * 128],
                    wv_sb[kc],
                    start=(kc == 0),
                    stop=(kc == 2),
                )
            va3 = va_mb[sub][st].rearrange("p (h c) -> p h c", c=128)
            src3 = ps.rearrange("p (h e) -> p h e", e=64)
            if st == 0:
                nc.scalar.copy(va3[:, :, 0:64], src3)
            else:
                nc.vector.tensor_copy(va3[:, :, 0:64], src3)

    def emit_scores(g):
        mb, sub, hp = stage(g)
        qk = qk_d[(mb, hp)]
        ps = ps_pair.tile([128, 1024], F32, tag="pair", name="s_ps")
        pair_d[g] = ps
        toff = sub * T
        for hh in range(2):
            rows = slice(hh * 64, (hh + 1) * 64)
            base = hh * 512
            # s-chunk 0: full t (N=256); s-chunk 1: only t >= 128 (N=128)
            nc.tensor.matmul(
                ps[:, base : base + 256],
                qk[rows, 512 + toff : 512 + toff + 128],
                qk[rows, toff : toff + 256],
                start=True,
                stop=True,
            )
            nc.tensor.matmul(
                ps[:, base + 256 : base + 384],
                qk[rows, 512 + toff + 128 : 512 + toff + 256],
                qk[rows, toff + 128 : toff + 256],
                start=True,
                stop=True,
            )

    def emit_exp(g):
        p = ppool.tile([128, 768], F16, tag="p", name="p_sb")
        in3 = pair_d.pop(g).rearrange("p (a c) -> p a c", c=512)[:, :, 0:384]
        nc.scalar.activation(
            p.rearrange("p (a c) -> p a c", c=384), in3, EXP, scale=SCALE
        )
        p_d[g] = p

    def emit_pmask(g):
        # zero the causally-masked halves of the two diagonal blocks per head
        # (post-exp, fp16, SBUF): one strided Pool multiply by a 0/1 mask
        p4 = p_d[g].rearrange("p (a b c) -> p a b c", b=3, c=128)[:, :, 0:3:2, :]
        nc.vector.tensor_mul(p4, p4, tri4v)

    def emit_pv(g):
        mb, sub, hp = stage(g)
        va_pair = va_all[mb % 2][sub]
        p3 = p_d.pop(g).rearrange("p (a c) -> p a c", c=384)
        o = ps_small.tile([128, 512], F32, tag="small", name="o_ps")
        o_d[g] = o
        for hh in range(2):
            h = hp * 2 + hh
            o_h = o[:, hh * 256 : (hh + 1) * 256]
            nc.tensor.matmul(
                o_h,
                va_pair[0][:, h * 128 : (h + 1) * 128],
                p3[:, hh, 0:256],
                start=True,
                stop=False,
            )
            nc.tensor.matmul(
                o_h[:, 128:256],
                va_pair[1][:, h * 128 : (h + 1) * 128],
                p3[:, hh, 256:384],
                start=False,
                stop=True,
            )

    def emit_recip(g):
        # reciprocal over the whole O_aug tile: rows 64:128 hold the
        # PE-replicated denominators (extra V_aug ones-columns), so no
        # partition broadcast is needed; rows 0:64 are unused. Base
        # partition stays 0 (the custom DVE op mishandles nonzero bases).
        rr = rrpool.tile([128, 512], F32, tag="rr", name="rr")
        nc.vector.reciprocal_approx_fast(rr, o_d[g])
        rr_d[g] = rr

    def emit_norm(g):
        mb, sub, hp = stage(g)
        rr = rr_d.pop(g)
        o = o_d.pop(g)
        if hp == 0:
            oT_d[(mb, sub)] = [None, None, None]
        oT = opool.tile([128, T], F16, tag="oT", name="oT")
        oT_d[(mb, sub)][hp] = oT
        for hh in range(2):
            nc.vector.tensor_mul(
                oT[hh * 64 : (hh + 1) * 64, :],
                o[0:64, hh * 256 : (hh + 1) * 256],
                rr[64:128, hh * 256 : (hh + 1) * 256],
            )

    def emit_outproj(mb, sub):
        ib = 2 * mb + sub
        oT = oT_d.pop((mb, sub))
        y_sb = ypool.tile([128, 2 * D], F32, tag="ysb", name="y_sb")
        for tt in range(2):
            yp = ps_small.tile([128, D], F32, tag="small", name="y_ps")
            for kc in range(3):
                nc.tensor.matmul(
                    yp,
                    oT[kc][:, tt * 128 : (tt + 1) * 128],
                    wo_sb[kc],
                    start=(kc == 0),
                    stop=(kc == 2),
                )
            nc.scalar.copy(y_sb[:, tt * D : (tt + 1) * D], yp)
        nc.sync.dma_start(
            out=y[ib].rearrange("(a t) d -> t a d", a=2),
            in_=y_sb.rearrange("p (a d) -> p a d", d=D),
        )

    # ---- slot pipeline ----
    emit_xload(0)
    nstages = nmb * 6
    for i in range(-2, nstages + 5):
        if stage(i - 3) is not None:
            emit_pv(i - 3)
            emit_recip(i - 3)
            emit_norm(i - 3)
        s_cur = stage(i)
        if s_cur is not None:
            emit_scores(i)
            emit_exp(i)
        # projections for macro-batch m are hoisted: q/k chunks two slots
        # ahead of m's first stage, v right at it, so copies never gate PE
        if i % 6 == 4 and stage(i + 2) is not None:
            m = i // 6 + 1
            emit_qkproj(m, 0)
            emit_qkproj(m, 1)
        if i % 6 == 5 and stage(i + 1) is not None:
            emit_qkproj(i // 6 + 1, 2)
        if s_cur is not None and i % 6 == 0:
            mb = i // 6
            emit_vproj(mb, 0)
            emit_xload(mb + 1)
            x_d.pop(mb - 1, None)
        if s_cur is not None and i % 6 == 1:
            emit_vproj(i // 6, 1)
        if s_cur is not None:
            emit_pmask(i)
        if stage(i - 4) is not None:
            mb4, sub4, hp4 = stage(i - 4)
            if hp4 == 2:
                emit_outproj(mb4, sub4)


def build_nc(bpc=BPC):
    nc = bacc.Bacc(
        "TRN2", target_bir_lowering=False, debug=False, enable_asserts=False
    )
    xT = nc.dram_tensor("xT", [bpc, D, T], F16, kind="ExternalInput").ap()
    wq = nc.dram_tensor("wq", [D, D], F16, kind="ExternalInput").ap()
    wk = nc.dram_tensor("wk", [D, D], F16, kind="ExternalInput").ap()
    wv = nc.dram_tensor("wv", [D, D], F16, kind="ExternalInput").ap()
    wo = nc.dram_tensor("wo", [D, D], F16, kind="ExternalInput").ap()
    tri4 = nc.dram_tensor("tri4", [128, 512], F16, kind="ExternalInput").ap()
    vone = nc.dram_tensor("vone", [128, H * 64], F16, kind="ExternalInput").ap()
    y = nc.dram_tensor("y", [bpc, T, D], F32, kind="ExternalOutput").ap()
    with tile.TileContext(nc) as tc:
        with ExitStack() as ctx:
            _emit(ctx, tc, (xT, wq, wk, wv, wo, tri4, vone, y), bpc)
    nc.finalize()
    return nc


_NC_CACHE = {}


def _get_nc(bpc):
    if bpc not in _NC_CACHE:
        _NC_CACHE[bpc] = build_nc(bpc)
    return _NC_CACHE[bpc]


def prep_inputs(x, W_qkv, W_out, b_out):
    x = np.asarray(x, np.float32)
    W_qkv = np.asarray(W_qkv, np.float32)
    nb = x.shape[0]
    xT = np.ascontiguousarray(x.transpose(0, 2, 1)).astype(np.float16)
    Wq = np.ascontiguousarray(
        W_qkv[:, :, 0:64].transpose(1, 0, 2).reshape(D, D)
    ).astype(np.float16)
    Wk = np.ascontiguousarray(
        W_qkv[:, :, 64:128].transpose(1, 0, 2).reshape(D, D)
    ).astype(np.float16)
    Wv = np.ascontiguousarray(
        W_qkv[:, :, 128:192].transpose(1, 0, 2).reshape(D, D)
    ).astype(np.float16)
    Wo = np.ascontiguousarray(np.asarray(W_out, np.float32)).astype(np.float16)
    tri01 = (np.arange(128)[:, None] <= np.arange(128)[None, :]).astype(np.float16)
    tri4 = np.tile(tri01, (1, 4))
    return xT, Wq, Wk, Wv, Wo, tri4, nb


def run(x, W_qkv, W_out, b_out, trace=False, **spmd_kwargs):
    xT, Wq, Wk, Wv, Wo, tri4, nb = prep_inputs(x, W_qkv, W_out, b_out)
    bpc = nb // NCORES
    assert bpc * NCORES == nb
    nc = _get_nc(bpc)
    shards = xT.reshape(NCORES, bpc, D, T)
    in_maps = [
        {
            "xT": shards[i],
            "wq": Wq,
            "wk": Wk,
            "wv": Wv,
            "wo": Wo,
            "tri4": tri4,
            "vone": np.ones((128, H * 64), np.float16),
        }
        for i in range(NCORES)
    ]
    res = run_bass_kernel_spmd(
        nc, in_maps, list(range(NCORES)), trace=trace, **spmd_kwargs
    )
    y = np.concatenate([res.results[i]["y"] for i in range(NCORES)], axis=0)
    b = np.asarray(b_out, np.float32)
    if b.any():
        y = y + b  # bias applied host-side (zeros in practice)
    return y, res


def kernel(x, W_qkv, W_out, b_out):
    y, _ = run(np.asarray(x), np.asarray(W_qkv), np.asarray(W_out), np.asarray(b_out))
    return y


# revision 32
# speedup vs baseline: 1.0333x; 1.0333x over previous
"""Trainium2 Bass kernel: 6-head causal self-attention (nn_MultiHead).

Strategy: pure data-parallel over batch B=256 across 8 NeuronCores
(32 batches/core, no collectives). Per batch, on-chip layout keeps the
contraction dim on SBUF partitions everywhere:

  host:    x [B,T,D] -> xT [B,D,T] fp16;  W_qkv -> Wq/Wk/Wv [D, H*HS] fp16;
           bias is applied host-side (zeros in practice)
  proj:    qT/kT [(h e), t] = W.T @ xT    (PE; two batches fused, N=512;
           q and k of one 128-row chunk share a 2-bank PSUM tile and one
           ACT copy)
  scores:  S^T [s, t] = kT_h^T @ qT_h per head (K=64), both heads of a
           pair in one 2-bank PSUM tile; causal skip: the s-chunk-1
           matmul only covers t >= 128 (N=128)
  softmax: P = exp(S/8), one paired ACT op [128,2,384] -> fp16 SBUF; the
           causal mask is applied POST-exp as a 0/1 multiply on the
           diagonal blocks (DVE, fp16, one strided op per stage)
  PV:      O_aug [128, t] = V_aug^T @ P^T where V_aug carries 64 extra
           all-ones columns, so rows 64:128 of the PSUM result hold the
           softmax denominators PRE-REPLICATED across 64 partitions (the
           extra matmul rows are free: PE cost depends only on N)
  norm:    fast reciprocal over the whole O_aug tile (DVE custom op
           reading PSUM; base partition must be 0), then
           O^T *= rr[64:128] while copying PSUM->SBUF (DVE) -- no
           partition broadcast or gather anywhere
  out:     y [t, d] = O^T.T @ W_out (PE), ACT copy PSUM->SBUF, one DMA
           per batch

A slot pipeline runs the six (sub-batch, head-pair) stages per
macro-batch: slot i emits PV/recip/normalize(i-3), scores+exp(i),
P-mask(i), out-projection(i-4); q/k projections are hoisted two slots
ahead of each macro-batch boundary and v projections split across its
first two slots, so the PE never drains and PSUM stays within the
8-bank budget (2x2-bank score/proj pairs + 4 single banks). Matmul
operands are fp16 (1 cycle/row streaming); accumulation is fp32 PSUM.
"""

import sys

import numpy as np

if "/opt/trn_rl_repo" not in sys.path:
    sys.path.insert(0, "/opt/trn_rl_repo")

from contextlib import ExitStack

import concourse.bass as bass  # noqa: F401
import concourse.tile as tile
from concourse import bacc, mybir
from concourse.bass_utils import run_bass_kernel_spmd

B, T, D, H, HS = 256, 256, 384, 6, 64
NCORES = 8
BPC = B // NCORES  # batches per core
F32 = mybir.dt.float32
F16 = mybir.dt.float16
EXP = mybir.ActivationFunctionType.Exp
SCALE = 1.0 / 8.0  # 1/sqrt(HS)


def _emit(ctx, tc, aps, bpc):
    nc = tc.nc
    xT, wq, wk, wv, wo, tri4, vone, y = aps
    assert bpc % 2 == 0
    nmb = bpc // 2  # macro-batches of 2

    singles = ctx.enter_context(tc.tile_pool(name="singles", bufs=1))
    xpool = ctx.enter_context(tc.tile_pool(name="xp", bufs=6))
    qkpool = ctx.enter_context(tc.tile_pool(name="qkp", bufs=6))
    ppool = ctx.enter_context(tc.tile_pool(name="pp", bufs=6))
    opool = ctx.enter_context(tc.tile_pool(name="op", bufs=12))
    rrpool = ctx.enter_context(tc.tile_pool(name="rrp", bufs=3))
    ypool = ctx.enter_context(tc.tile_pool(name="yp", bufs=4))
    ps_pair = ctx.enter_context(tc.tile_pool(name="ps_pair", bufs=2, space="PSUM"))
    ps_small = ctx.enter_context(tc.tile_pool(name="ps_small", bufs=4, space="PSUM"))

    # Constants / weights, loaded once.
    def _load(name, src, shape, dt=F16):
        t = singles.tile(shape, dt, tag=name, name=name)
        nc.sync.dma_start(out=t, in_=src)
        return t

    wq_sb = [None, None, None]
    wk_sb = [None, None, None]
    wv_sb = [None, None, None]
    wo_sb = [None, None, None]

    def _load_weights():
        # emitted after the first x-load so the Sync queue serves the
        # pipeline-critical transfers in first-use order
        for i in range(3):
            wq_sb[i] = _load(f"wq{i}", wq[i * 128 : (i + 1) * 128, :], [128, D])
            wk_sb[i] = _load(f"wk{i}", wk[i * 128 : (i + 1) * 128, :], [128, D])
        for i in range(3):
            wv_sb[i] = _load(f"wv{i}", wv[i * 128 : (i + 1) * 128, :], [128, D])
        for i in range(3):
            wo_sb[i] = _load(f"wo{i}", wo[i * 128 : (i + 1) * 128, :], [128, D])

    tri4_sb = _load("tri4", tri4, [128, 512])
    tri4v = tri4_sb.rearrange("p (a b c) -> p a b c", b=2, c=128)

    # Persistent v_aug tiles [macro-parity][sub-batch][s-tile]: ones columns
    # are DMA'd once and survive all batches (the per-batch copy writes only
    # cols 0:64 of each 65-wide head block).
    va_all = []

    def _load_va():
        for par in range(2):
            subs = []
            for sub in range(2):
                pair = []
                for st in range(2):
                    t = singles.tile(
                        [128, H * 128],
                        F16,
                        tag=f"va{par}{sub}{st}",
                        name=f"va{par}{sub}{st}",
                    )
                    nc.sync.dma_start(
                        out=t.rearrange("p (h c) -> p h c", c=128)[:, :, 64:128],
                        in_=vone.rearrange("p (h c) -> p h c", c=64),
                    )
                    pair.append(t)
                subs.append(pair)
            va_all.append(subs)

    # xT viewed so two consecutive batches concatenate along the free dim:
    # [mb, d, (sub t)] per 128-row d-chunk
    def x2_src(mb, kc):
        return xT[2 * mb : 2 * mb + 2, kc * 128 : (kc + 1) * 128, :].rearrange(
            "b d t -> d b t"
        )

    # Pipeline state, keyed by global stage index g = mb*6 + sub*3 + hp.
    x_d = {}  # mb -> [3 x-tiles]
    qk_d = {}  # (mb, mt) -> [128, 1024] f16 (q cols 0:512, k cols 512:1024)
    pair_d = {}  # g -> [128, 1024] f32 psum (scores, both heads)
    p_d = {}  # g -> [128, 768] f16 (exp output, both heads)
    o_d = {}  # g -> [128, 512] f32 psum (PV out; rows 64:128 = denominators)
    rr_d = {}  # g -> [128, 512] f32 (rows 64:128 = denominator reciprocals)
    oT_d = {}  # (mb, sub) -> [3 oT tiles]

    def stage(i):
        if 0 <= i < nmb * 6:
            return i // 6, (i % 6) // 3, i % 3  # mb, sub, hp
        return None

    def emit_xload(mb):
        if mb >= nmb:
            return
        tiles = []
        for kc in range(3):
            t = xpool.tile([128, 2 * T], F16, tag="x", name="x")
            nc.sync.dma_start(
                out=t.rearrange("p (b t) -> p b t", t=T), in_=x2_src(mb, kc)
            )
            tiles.append(t)
        x_d[mb] = tiles

    def emit_qkproj(mb, mt):
        x_sb = x_d[mb]
        ps = ps_pair.tile([128, 1024], F32, tag="pair", name="qk_ps")
        for kc in range(3):
            nc.tensor.matmul(
                ps[:, 0:512],
                wq_sb[kc][:, mt * 128 : (mt + 1) * 128],
                x_sb[kc],
                start=(kc == 0),
                stop=(kc == 2),
            )
        for kc in range(3):
            nc.tensor.matmul(
                ps[:, 512:1024],
                wk_sb[kc][:, mt * 128 : (mt + 1) * 128],
                x_sb[kc],
                start=(kc == 0),
                stop=(kc == 2),
            )
        sb = qkpool.tile([128, 1024], F16, tag="qk", name="qk_sb")
        nc.scalar.copy(sb, ps)
        qk_d[(mb, mt)] = sb

    def emit_vproj(mb, sub):
        x_sb = x_d[mb]
        va_mb = va_all[mb % 2]
        for st in range(2):
            ps = ps_small.tile([128, D], F32, tag="small", name="v_ps")
            for kc in range(3):
                nc.tensor.matmul(
                    ps,
                    x_sb[kc][:, sub * T + st * 128 : sub * T + (st + 1) * 128],
                    wv_sb[kc],
                    start=(kc == 0),
                    stop=(kc == 2),
                )
            va3 = va_mb[sub][st].rearrange("p (h c) -> p h c", c=128)
            src3 = ps.rearrange("p (h e) -> p h e", e=64)
            if st == 0:
                nc.scalar.copy(va3[:, :, 0:64], src3)
            else:
                nc.vector.tensor_copy(va3[:, :, 0:64], src3)

    def emit_scores(g):
        mb, sub, hp = stage(g)
        qk = qk_d[(mb, hp)]
        ps = ps_pair.tile([128, 1024], F32, tag="pair", name="s_ps")
        pair_d[g] = ps
        toff = sub * T
        for hh in range(2):
            rows = slice(hh * 64, (hh + 1) * 64)
            base = hh * 512
            # s-chunk 0: full t (N=256); s-chunk 1: only t >= 128 (N=128)
            nc.tensor.matmul(
                ps[:, base : base + 256],
                qk[rows, 512 + toff : 512 + toff + 128],
                qk[rows, toff : toff + 256],
                start=True,
                stop=True,
            )
            nc.tensor.matmul(
                ps[:, base + 256 : base + 384],
                qk[rows, 512 + toff + 128 : 512 + toff + 256],
                qk[rows, toff + 128 : toff + 256],
                start=True,
                stop=True,
            )

    def emit_exp(g):
        p = ppool.tile([128, 768], F16, tag="p", name="p_sb")
        in3 = pair_d.pop(g).rearrange("p (a c) -> p a c", c=512)[:, :, 0:384]
        nc.scalar.activation(
            p.rearrange("p (a c) -> p a c", c=384), in3, EXP, scale=SCALE
        )
        p_d[g] = p

    def emit_pmask(g):
        # zero the causally-masked halves of the two diagonal blocks per head
        # (post-exp, fp16, SBUF): one strided Pool multiply by a 0/1 mask
        p4 = p_d[g].rearrange("p (a b c) -> p a b c", b=3, c=128)[:, :, 0:3:2, :]
        nc.vector.tensor_mul(p4, p4, tri4v)

    def emit_pv(g):
        mb, sub, hp = stage(g)
        va_pair = va_all[mb % 2][sub]
        p3 = p_d.pop(g).rearrange("p (a c) -> p a c", c=384)
        o = ps_small.tile([128, 512], F32, tag="small", name="o_ps")
        o_d[g] = o
        for hh in range(2):
            h = hp * 2 + hh
            o_h = o[:, hh * 256 : (hh + 1) * 256]
            nc.tensor.matmul(
                o_h,
                va_pair[0][:, h * 128 : (h + 1) * 128],
                p3[:, hh, 0:256],
                start=True,
                stop=False,
            )
            nc.tensor.matmul(
                o_h[:, 128:256],
                va_pair[1][:, h * 128 : (h + 1) * 128],
                p3[:, hh, 256:384],
                start=False,
                stop=True,
            )

    def emit_recip(g):
        # reciprocal over the whole O_aug tile: rows 64:128 hold the
        # PE-replicated denominators (extra V_aug ones-columns), so no
        # partition broadcast is needed; rows 0:64 are unused. Base
        # partition stays 0 (the custom DVE op mishandles nonzero bases).
        rr = rrpool.tile([128, 512], F32, tag="rr", name="rr")
        nc.vector.reciprocal_approx_fast(rr, o_d[g])
        rr_d[g] = rr

    def emit_norm(g):
        mb, sub, hp = stage(g)
        rr = rr_d.pop(g)
        o = o_d.pop(g)
        if hp == 0:
            oT_d[(mb, sub)] = [None, None, None]
        oT = opool.tile([128, T], F16, tag="oT", name="oT")
        oT_d[(mb, sub)][hp] = oT
        for hh in range(2):
            nc.vector.tensor_mul(
                oT[hh * 64 : (hh + 1) * 64, :],
                o[0:64, hh * 256 : (hh + 1) * 256],
                rr[64:128, hh * 256 : (hh + 1) * 256],
            )

    def emit_outproj(mb, sub):
        ib = 2 * mb + sub
        oT = oT_d.pop((mb, sub))
        y_sb = ypool.tile([128, 2 * D], F32, tag="ysb", name="y_sb")
        for tt in range(2):
            yp = ps_small.tile([128, D], F32, tag="small", name="y_ps")
            for kc in range(3):
                nc.tensor.matmul(
                    yp,
                    oT[kc][:, tt * 128 : (tt + 1) * 128],
                    wo_sb[kc],
                    start=(kc == 0),
                    stop=(kc == 2),
                )
            nc.scalar.copy(y_sb[:, tt * D : (tt + 1) * D], yp)
        nc.sync.dma_start(
            out=y[ib].rearrange("(a t) d -> t a d", a=2),
            in_=y_sb.rearrange("p (a d) -> p a d", d=D),
        )

    # ---- slot pipeline ----
    emit_xload(0)
    _load_weights()
    _load_va()
    nstages = nmb * 6
    for i in range(-2, nstages + 5):
        if stage(i - 3) is not None:
            emit_pv(i - 3)
            emit_recip(i - 3)
            emit_norm(i - 3)
        s_cur = stage(i)
        if s_cur is not None:
            emit_scores(i)
            emit_exp(i)
        # projections for macro-batch m are hoisted: q/k chunks two slots
        # ahead of m's first stage, v right at it, so copies never gate PE
        if i % 6 == 4 and stage(i + 2) is not None:
            m = i // 6 + 1
            emit_qkproj(m, 0)
            emit_qkproj(m, 1)
        if i % 6 == 5 and stage(i + 1) is not None:
            emit_qkproj(i // 6 + 1, 2)
        if s_cur is not None and i % 6 == 0:
            mb = i // 6
            emit_vproj(mb, 0)
            emit_xload(mb + 1)
            x_d.pop(mb - 1, None)
        if s_cur is not None and i % 6 == 1:
            emit_vproj(i // 6, 1)
        if s_cur is not None:
            emit_pmask(i)
        if stage(i - 4) is not None:
            mb4, sub4, hp4 = stage(i - 4)
            if hp4 == 2:
                emit_outproj(mb4, sub4)


def build_nc(bpc=BPC):
    nc = bacc.Bacc(
        "TRN2", target_bir_lowering=False, debug=False, enable_asserts=False
    )
    xT = nc.dram_tensor("xT", [bpc, D, T], F16, kind="ExternalInput").ap()
    wq = nc.dram_tensor("wq", [D, D], F16, kind="ExternalInput").ap()
    wk = nc.dram_tensor("wk", [D, D], F16, kind="ExternalInput").ap()
    wv = nc.dram_tensor("wv", [D, D], F16, kind="ExternalInput").ap()
    wo = nc.dram_tensor("wo", [D, D], F16, kind="ExternalInput").ap()
    tri4 = nc.dram_tensor("tri4", [128, 512], F16, kind="ExternalInput").ap()
    vone = nc.dram_tensor("vone", [128, H * 64], F16, kind="ExternalInput").ap()
    y = nc.dram_tensor("y", [bpc, T, D], F32, kind="ExternalOutput").ap()
    with tile.TileContext(nc) as tc:
        with ExitStack() as ctx:
            _emit(ctx, tc, (xT, wq, wk, wv, wo, tri4, vone, y), bpc)
    nc.finalize()
    return nc


_NC_CACHE = {}


def _get_nc(bpc):
    if bpc not in _NC_CACHE:
        _NC_CACHE[bpc] = build_nc(bpc)
    return _NC_CACHE[bpc]


def prep_inputs(x, W_qkv, W_out, b_out):
    x = np.asarray(x, np.float32)
    W_qkv = np.asarray(W_qkv, np.float32)
    nb = x.shape[0]
    xT = np.ascontiguousarray(x.transpose(0, 2, 1)).astype(np.float16)
    Wq = np.ascontiguousarray(
        W_qkv[:, :, 0:64].transpose(1, 0, 2).reshape(D, D)
    ).astype(np.float16)
    Wk = np.ascontiguousarray(
        W_qkv[:, :, 64:128].transpose(1, 0, 2).reshape(D, D)
    ).astype(np.float16)
    Wv = np.ascontiguousarray(
        W_qkv[:, :, 128:192].transpose(1, 0, 2).reshape(D, D)
    ).astype(np.float16)
    Wo = np.ascontiguousarray(np.asarray(W_out, np.float32)).astype(np.float16)
    tri01 = (np.arange(128)[:, None] <= np.arange(128)[None, :]).astype(np.float16)
    tri4 = np.tile(tri01, (1, 4))
    return xT, Wq, Wk, Wv, Wo, tri4, nb


def run(x, W_qkv, W_out, b_out, trace=False, **spmd_kwargs):
    xT, Wq, Wk, Wv, Wo, tri4, nb = prep_inputs(x, W_qkv, W_out, b_out)
    bpc = nb // NCORES
    assert bpc * NCORES == nb
    nc = _get_nc(bpc)
    shards = xT.reshape(NCORES, bpc, D, T)
    in_maps = [
        {
            "xT": shards[i],
            "wq": Wq,
            "wk": Wk,
            "wv": Wv,
            "wo": Wo,
            "tri4": tri4,
            "vone": np.ones((128, H * 64), np.float16),
        }
        for i in range(NCORES)
    ]
    res = run_bass_kernel_spmd(
        nc, in_maps, list(range(NCORES)), trace=trace, **spmd_kwargs
    )
    y = np.concatenate([res.results[i]["y"] for i in range(NCORES)], axis=0)
    b = np.asarray(b_out, np.float32)
    if b.any():
        y = y + b  # bias applied host-side (zeros in practice)
    return y, res


def kernel(x, W_qkv, W_out, b_out):
    y, _ = run(np.asarray(x), np.asarray(W_qkv), np.asarray(W_out), np.asarray(b_out))
    return y


# revision 33
# speedup vs baseline: 1.0373x; 1.0039x over previous
"""Trainium2 Bass kernel: 6-head causal self-attention (nn_MultiHead).

Strategy: pure data-parallel over batch B=256 across 8 NeuronCores
(32 batches/core, no collectives). Per batch, on-chip layout keeps the
contraction dim on SBUF partitions everywhere:

  host:    x [B,T,D] -> xT [B,D,T] fp16;  W_qkv -> Wq/Wk/Wv [D, H*HS] fp16;
           bias is applied host-side (zeros in practice)
  proj:    qT/kT [(h e), t] = W.T @ xT    (PE; two batches fused, N=512;
           q and k of one 128-row chunk share a 2-bank PSUM tile and one
           ACT copy)
  scores:  S^T [s, t] = kT_h^T @ qT_h per head (K=64), both heads of a
           pair in one 2-bank PSUM tile; causal skip: the s-chunk-1
           matmul only covers t >= 128 (N=128)
  softmax: P = exp(S/8), one paired ACT op [128,2,384] -> fp16 SBUF; the
           causal mask is applied POST-exp as a 0/1 multiply on the
           diagonal blocks (DVE, fp16, one strided op per stage)
  PV:      O_aug [128, t] = V_aug^T @ P^T where V_aug carries 64 extra
           all-ones columns, so rows 64:128 of the PSUM result hold the
           softmax denominators PRE-REPLICATED across 64 partitions (the
           extra matmul rows are free: PE cost depends only on N)
  norm:    fast reciprocal over the whole O_aug tile (DVE custom op
           reading PSUM; base partition must be 0), then
           O^T *= rr[64:128] while copying PSUM->SBUF (DVE) -- no
           partition broadcast or gather anywhere
  out:     y [t, d] = O^T.T @ W_out (PE), ACT copy PSUM->SBUF, one DMA
           per batch

A slot pipeline runs the six (sub-batch, head-pair) stages per
macro-batch: slot i emits PV/recip/normalize(i-3), scores+exp(i),
P-mask(i), out-projection(i-4); q/k projections are hoisted two slots
ahead of each macro-batch boundary and v projections split across its
first two slots, so the PE never drains and PSUM stays within the
8-bank budget (2x2-bank score/proj pairs + 4 single banks). Matmul
operands are fp16 (1 cycle/row streaming); accumulation is fp32 PSUM.
"""

import sys

import numpy as np

if "/opt/trn_rl_repo" not in sys.path:
    sys.path.insert(0, "/opt/trn_rl_repo")

from contextlib import ExitStack

import concourse.bass as bass  # noqa: F401
import concourse.tile as tile
from concourse import bacc, mybir
from concourse.bass_utils import run_bass_kernel_spmd

B, T, D, H, HS = 256, 256, 384, 6, 64
NCORES = 8
BPC = B // NCORES  # batches per core
F32 = mybir.dt.float32
F16 = mybir.dt.float16
EXP = mybir.ActivationFunctionType.Exp
SCALE = 1.0 / 8.0  # 1/sqrt(HS)


def _emit(ctx, tc, aps, bpc):
    nc = tc.nc
    xT, wq, wk, wv, wo, tri4, vone, y = aps
    assert bpc % 2 == 0
    nmb = bpc // 2  # macro-batches of 2

    singles = ctx.enter_context(tc.tile_pool(name="singles", bufs=1))
    xpool = ctx.enter_context(tc.tile_pool(name="xp", bufs=6))
    qkpool = ctx.enter_context(tc.tile_pool(name="qkp", bufs=6))
    ppool = ctx.enter_context(tc.tile_pool(name="pp", bufs=6))
    opool = ctx.enter_context(tc.tile_pool(name="op", bufs=12))
    rrpool = ctx.enter_context(tc.tile_pool(name="rrp", bufs=3))
    ypool = ctx.enter_context(tc.tile_pool(name="yp", bufs=4))
    ps_pair = ctx.enter_context(tc.tile_pool(name="ps_pair", bufs=2, space="PSUM"))
    ps_small = ctx.enter_context(tc.tile_pool(name="ps_small", bufs=4, space="PSUM"))

    # Constants / weights, loaded once.
    def _load(name, src, shape, dt=F16):
        t = singles.tile(shape, dt, tag=name, name=name)
        nc.sync.dma_start(out=t, in_=src)
        return t

    wq_sb = [None, None, None]
    wk_sb = [None, None, None]
    wv_sb = [None, None, None]
    wo_sb = [None, None, None]

    def _load_weights():
        # emitted after the first x-load so the Sync queue serves the
        # pipeline-critical transfers in first-use order; q/k weights come
        # in one DMA each (all three 128-row chunks stacked on the free dim)
        wq_big = _load("wqb", wq.rearrange("(c p) d -> p c d", p=128), [128, 3 * D])
        wk_big = _load("wkb", wk.rearrange("(c p) d -> p c d", p=128), [128, 3 * D])
        for i in range(3):
            wq_sb[i] = wq_big.rearrange("p (c d) -> p c d", d=D)[:, i, :]
            wk_sb[i] = wk_big.rearrange("p (c d) -> p c d", d=D)[:, i, :]
        for i in range(3):
            wv_sb[i] = _load(f"wv{i}", wv[i * 128 : (i + 1) * 128, :], [128, D])
        for i in range(3):
            wo_sb[i] = _load(f"wo{i}", wo[i * 128 : (i + 1) * 128, :], [128, D])

    tri4_sb = _load("tri4", tri4, [128, 512])
    tri4v = tri4_sb.rearrange("p (a b c) -> p a b c", b=2, c=128)

    # Persistent v_aug tiles [macro-parity][sub-batch][s-tile]: ones columns
    # are DMA'd once and survive all batches (the per-batch copy writes only
    # cols 0:64 of each 65-wide head block).
    va_all = []

    def _load_va():
        for par in range(2):
            subs = []
            for sub in range(2):
                pair = []
                for st in range(2):
                    t = singles.tile(
                        [128, H * 128],
                        F16,
                        tag=f"va{par}{sub}{st}",
                        name=f"va{par}{sub}{st}",
                    )
                    nc.sync.dma_start(
                        out=t.rearrange("p (h c) -> p h c", c=128)[:, :, 64:128],
                        in_=vone.rearrange("p (h c) -> p h c", c=64),
                    )
                    pair.append(t)
                subs.append(pair)
            va_all.append(subs)

    # xT viewed so two consecutive batches concatenate along the free dim:
    # [mb, d, (sub t)] per 128-row d-chunk
    def x2_src(mb, kc):
        return xT[2 * mb : 2 * mb + 2, kc * 128 : (kc + 1) * 128, :].rearrange(
            "b d t -> d b t"
        )

    # Pipeline state, keyed by global stage index g = mb*6 + sub*3 + hp.
    x_d = {}  # mb -> [3 x-tiles]
    qk_d = {}  # (mb, mt) -> [128, 1024] f16 (q cols 0:512, k cols 512:1024)
    pair_d = {}  # g -> [128, 1024] f32 psum (scores, both heads)
    p_d = {}  # g -> [128, 768] f16 (exp output, both heads)
    o_d = {}  # g -> [128, 512] f32 psum (PV out; rows 64:128 = denominators)
    rr_d = {}  # g -> [128, 512] f32 (rows 64:128 = denominator reciprocals)
    oT_d = {}  # (mb, sub) -> [3 oT tiles]

    def stage(i):
        if 0 <= i < nmb * 6:
            return i // 6, (i % 6) // 3, i % 3  # mb, sub, hp
        return None

    def emit_xload(mb):
        if mb >= nmb:
            return
        eng = nc.scalar if mb == 0 else nc.sync
        tiles = []
        for kc in range(3):
            t = xpool.tile([128, 2 * T], F16, tag="x", name="x")
            eng.dma_start(
                out=t.rearrange("p (b t) -> p b t", t=T), in_=x2_src(mb, kc)
            )
            tiles.append(t)
        x_d[mb] = tiles

    def emit_qkproj(mb, mt):
        x_sb = x_d[mb]
        ps = ps_pair.tile([128, 1024], F32, tag="pair", name="qk_ps")
        for kc in range(3):
            nc.tensor.matmul(
                ps[:, 0:512],
                wq_sb[kc][:, mt * 128 : (mt + 1) * 128],
                x_sb[kc],
                start=(kc == 0),
                stop=(kc == 2),
            )
        for kc in range(3):
            nc.tensor.matmul(
                ps[:, 512:1024],
                wk_sb[kc][:, mt * 128 : (mt + 1) * 128],
                x_sb[kc],
                start=(kc == 0),
                stop=(kc == 2),
            )
        sb = qkpool.tile([128, 1024], F16, tag="qk", name="qk_sb")
        nc.scalar.copy(sb, ps)
        qk_d[(mb, mt)] = sb

    def emit_vproj(mb, sub):
        x_sb = x_d[mb]
        va_mb = va_all[mb % 2]
        for st in range(2):
            ps = ps_small.tile([128, D], F32, tag="small", name="v_ps")
            for kc in range(3):
                nc.tensor.matmul(
                    ps,
                    x_sb[kc][:, sub * T + st * 128 : sub * T + (st + 1) * 128],
                    wv_sb[kc],
                    start=(kc == 0),
                    stop=(kc == 2),
                )
            va3 = va_mb[sub][st].rearrange("p (h c) -> p h c", c=128)
            src3 = ps.rearrange("p (h e) -> p h e", e=64)
            if st == 0:
                nc.scalar.copy(va3[:, :, 0:64], src3)
            else:
                nc.vector.tensor_copy(va3[:, :, 0:64], src3)

    def emit_scores(g):
        mb, sub, hp = stage(g)
        qk = qk_d[(mb, hp)]
        ps = ps_pair.tile([128, 1024], F32, tag="pair", name="s_ps")
        pair_d[g] = ps
        toff = sub * T
        for hh in range(2):
            rows = slice(hh * 64, (hh + 1) * 64)
            base = hh * 512
            # s-chunk 0: full t (N=256); s-chunk 1: only t >= 128 (N=128)
            nc.tensor.matmul(
                ps[:, base : base + 256],
                qk[rows, 512 + toff : 512 + toff + 128],
                qk[rows, toff : toff + 256],
                start=True,
                stop=True,
            )
            nc.tensor.matmul(
                ps[:, base + 256 : base + 384],
                qk[rows, 512 + toff + 128 : 512 + toff + 256],
                qk[rows, toff + 128 : toff + 256],
                start=True,
                stop=True,
            )

    def emit_exp(g):
        p = ppool.tile([128, 768], F16, tag="p", name="p_sb")
        in3 = pair_d.pop(g).rearrange("p (a c) -> p a c", c=512)[:, :, 0:384]
        nc.scalar.activation(
            p.rearrange("p (a c) -> p a c", c=384), in3, EXP, scale=SCALE
        )
        p_d[g] = p

    def emit_pmask(g):
        # zero the causally-masked halves of the two diagonal blocks per head
        # (post-exp, fp16, SBUF): one strided Pool multiply by a 0/1 mask
        p4 = p_d[g].rearrange("p (a b c) -> p a b c", b=3, c=128)[:, :, 0:3:2, :]
        nc.vector.tensor_mul(p4, p4, tri4v)

    def emit_pv(g):
        mb, sub, hp = stage(g)
        va_pair = va_all[mb % 2][sub]
        p3 = p_d.pop(g).rearrange("p (a c) -> p a c", c=384)
        o = ps_small.tile([128, 512], F32, tag="small", name="o_ps")
        o_d[g] = o
        for hh in range(2):
            h = hp * 2 + hh
            o_h = o[:, hh * 256 : (hh + 1) * 256]
            nc.tensor.matmul(
                o_h,
                va_pair[0][:, h * 128 : (h + 1) * 128],
                p3[:, hh, 0:256],
                start=True,
                stop=False,
            )
            nc.tensor.matmul(
                o_h[:, 128:256],
                va_pair[1][:, h * 128 : (h + 1) * 128],
                p3[:, hh, 256:384],
                start=False,
                stop=True,
            )

    def emit_recip(g):
        # reciprocal over the whole O_aug tile: rows 64:128 hold the
        # PE-replicated denominators (extra V_aug ones-columns), so no
        # partition broadcast is needed; rows 0:64 are unused. Base
        # partition stays 0 (the custom DVE op mishandles nonzero bases).
        rr = rrpool.tile([128, 512], F32, tag="rr", name="rr")
        nc.vector.reciprocal_approx_fast(rr, o_d[g])
        rr_d[g] = rr

    def emit_norm(g):
        mb, sub, hp = stage(g)
        rr = rr_d.pop(g)
        o = o_d.pop(g)
        if hp == 0:
            oT_d[(mb, sub)] = [None, None, None]
        oT = opool.tile([128, T], F16, tag="oT", name="oT")
        oT_d[(mb, sub)][hp] = oT
        for hh in range(2):
            nc.vector.tensor_mul(
                oT[hh * 64 : (hh + 1) * 64, :],
                o[0:64, hh * 256 : (hh + 1) * 256],
                rr[64:128, hh * 256 : (hh + 1) * 256],
            )

    def emit_outproj(mb, sub):
        ib = 2 * mb + sub
        oT = oT_d.pop((mb, sub))
        y_sb = ypool.tile([128, 2 * D], F32, tag="ysb", name="y_sb")
        for tt in range(2):
            yp = ps_small.tile([128, D], F32, tag="small", name="y_ps")
            for kc in range(3):
                nc.tensor.matmul(
                    yp,
                    oT[kc][:, tt * 128 : (tt + 1) * 128],
                    wo_sb[kc],
                    start=(kc == 0),
                    stop=(kc == 2),
                )
            nc.scalar.copy(y_sb[:, tt * D : (tt + 1) * D], yp)
        nc.sync.dma_start(
            out=y[ib].rearrange("(a t) d -> t a d", a=2),
            in_=y_sb.rearrange("p (a d) -> p a d", d=D),
        )

    # ---- slot pipeline ----
    emit_xload(0)
    _load_weights()
    _load_va()
    nstages = nmb * 6
    for i in range(-2, nstages + 5):
        if stage(i - 3) is not None:
            emit_pv(i - 3)
            emit_recip(i - 3)
            emit_norm(i - 3)
        s_cur = stage(i)
        if s_cur is not None:
            emit_scores(i)
            emit_exp(i)
        # projections for macro-batch m are hoisted: q/k chunks two slots
        # ahead of m's first stage, v right at it, so copies never gate PE
        if i % 6 == 4 and stage(i + 2) is not None:
            m = i // 6 + 1
            emit_qkproj(m, 0)
            emit_qkproj(m, 1)
        if i % 6 == 5 and stage(i + 1) is not None:
            emit_qkproj(i // 6 + 1, 2)
        if s_cur is not None and i % 6 == 0:
            mb = i // 6
            emit_vproj(mb, 0)
            emit_xload(mb + 1)
            x_d.pop(mb - 1, None)
        if s_cur is not None and i % 6 == 1:
            emit_vproj(i // 6, 1)
        if s_cur is not None:
            emit_pmask(i)
        if stage(i - 3) is not None and i - 3 == nstages - 1:
            emit_outproj(nmb - 1, 1)
        if stage(i - 4) is not None:
            mb4, sub4, hp4 = stage(i - 4)
            if hp4 == 2 and (mb4, sub4) in oT_d:
                emit_outproj(mb4, sub4)


def build_nc(bpc=BPC):
    nc = bacc.Bacc(
        "TRN2", target_bir_lowering=False, debug=False, enable_asserts=False
    )
    xT = nc.dram_tensor("xT", [bpc, D, T], F16, kind="ExternalInput").ap()
    wq = nc.dram_tensor("wq", [D, D], F16, kind="ExternalInput").ap()
    wk = nc.dram_tensor("wk", [D, D], F16, kind="ExternalInput").ap()
    wv = nc.dram_tensor("wv", [D, D], F16, kind="ExternalInput").ap()
    wo = nc.dram_tensor("wo", [D, D], F16, kind="ExternalInput").ap()
    tri4 = nc.dram_tensor("tri4", [128, 512], F16, kind="ExternalInput").ap()
    vone = nc.dram_tensor("vone", [128, H * 64], F16, kind="ExternalInput").ap()
    y = nc.dram_tensor("y", [bpc, T, D], F32, kind="ExternalOutput").ap()
    with tile.TileContext(nc) as tc:
        with ExitStack() as ctx:
            _emit(ctx, tc, (xT, wq, wk, wv, wo, tri4, vone, y), bpc)
    nc.finalize()
    return nc


_NC_CACHE = {}


def _get_nc(bpc):
    if bpc not in _NC_CACHE:
        _NC_CACHE[bpc] = build_nc(bpc)
    return _NC_CACHE[bpc]


def prep_inputs(x, W_qkv, W_out, b_out):
    x = np.asarray(x, np.float32)
    W_qkv = np.asarray(W_qkv, np.float32)
    nb = x.shape[0]
    xT = np.ascontiguousarray(x.transpose(0, 2, 1)).astype(np.float16)
    Wq = np.ascontiguousarray(
        W_qkv[:, :, 0:64].transpose(1, 0, 2).reshape(D, D)
    ).astype(np.float16)
    Wk = np.ascontiguousarray(
        W_qkv[:, :, 64:128].transpose(1, 0, 2).reshape(D, D)
    ).astype(np.float16)
    Wv = np.ascontiguousarray(
        W_qkv[:, :, 128:192].transpose(1, 0, 2).reshape(D, D)
    ).astype(np.float16)
    Wo = np.ascontiguousarray(np.asarray(W_out, np.float32)).astype(np.float16)
    tri01 = (np.arange(128)[:, None] <= np.arange(128)[None, :]).astype(np.float16)
    tri4 = np.tile(tri01, (1, 4))
    return xT, Wq, Wk, Wv, Wo, tri4, nb


def run(x, W_qkv, W_out, b_out, trace=False, **spmd_kwargs):
    xT, Wq, Wk, Wv, Wo, tri4, nb = prep_inputs(x, W_qkv, W_out, b_out)
    bpc = nb // NCORES
    assert bpc * NCORES == nb
    nc = _get_nc(bpc)
    shards = xT.reshape(NCORES, bpc, D, T)
    in_maps = [
        {
            "xT": shards[i],
            "wq": Wq,
            "wk": Wk,
            "wv": Wv,
            "wo": Wo,
            "tri4": tri4,
            "vone": np.ones((128, H * 64), np.float16),
        }
        for i in range(NCORES)
    ]
    res = run_bass_kernel_spmd(
        nc, in_maps, list(range(NCORES)), trace=trace, **spmd_kwargs
    )
    y = np.concatenate([res.results[i]["y"] for i in range(NCORES)], axis=0)
    b = np.asarray(b_out, np.float32)
    if b.any():
        y = y + b  # bias applied host-side (zeros in practice)
    return y, res


def kernel(x, W_qkv, W_out, b_out):
    y, _ = run(np.asarray(x), np.asarray(W_qkv), np.asarray(W_out), np.asarray(b_out))
    return y


# revision 34
# speedup vs baseline: 1.0446x; 1.0070x over previous
"""Trainium2 Bass kernel: 6-head causal self-attention (nn_MultiHead).

Strategy: pure data-parallel over batch B=256 across 8 NeuronCores
(32 batches/core, no collectives). Per batch, on-chip layout keeps the
contraction dim on SBUF partitions everywhere:

  host:    x [B,T,D] -> xT [B,D,T] fp16;  W_qkv -> Wq/Wk/Wv [D, H*HS] fp16;
           bias is applied host-side (zeros in practice)
  proj:    qT/kT [(h e), t] = W.T @ xT    (PE; two batches fused, N=512;
           q and k of one 128-row chunk share a 2-bank PSUM tile and one
           ACT copy)
  scores:  S^T [s, t] = kT_h^T @ qT_h per head (K=64), both heads of a
           pair in one 2-bank PSUM tile; causal skip: the s-chunk-1
           matmul only covers t >= 128 (N=128)
  softmax: P = exp(S/8), one paired ACT op [128,2,384] -> fp16 SBUF; the
           causal mask is applied POST-exp as a 0/1 multiply on the
           diagonal blocks (DVE, fp16, one strided op per stage)
  PV:      O_aug [128, t] = V_aug^T @ P^T where V_aug carries 64 extra
           all-ones columns, so rows 64:128 of the PSUM result hold the
           softmax denominators PRE-REPLICATED across 64 partitions (the
           extra matmul rows are free: PE cost depends only on N)
  norm:    fast reciprocal over the whole O_aug tile (DVE custom op
           reading PSUM; base partition must be 0), then
           O^T *= rr[64:128] while copying PSUM->SBUF (DVE) -- no
           partition broadcast or gather anywhere
  out:     y [t, d] = O^T.T @ W_out (PE), ACT copy PSUM->SBUF, one DMA
           per batch

A slot pipeline runs the six (sub-batch, head-pair) stages per
macro-batch: slot i emits PV/recip/normalize(i-3), scores+exp(i),
P-mask(i), out-projection(i-4); q/k projections are hoisted two slots
ahead of each macro-batch boundary and v projections split across its
first two slots, so the PE never drains and PSUM stays within the
8-bank budget (2x2-bank score/proj pairs + 4 single banks). Matmul
operands are fp16 (1 cycle/row streaming); accumulation is fp32 PSUM.
"""

import sys

import numpy as np

if "/opt/trn_rl_repo" not in sys.path:
    sys.path.insert(0, "/opt/trn_rl_repo")

from contextlib import ExitStack

import concourse.bass as bass  # noqa: F401
import concourse.tile as tile
from concourse import bacc, mybir
from concourse.bass_utils import run_bass_kernel_spmd

B, T, D, H, HS = 256, 256, 384, 6, 64
NCORES = 8
BPC = B // NCORES  # batches per core
F32 = mybir.dt.float32
F16 = mybir.dt.float16
EXP = mybir.ActivationFunctionType.Exp
SCALE = 1.0 / 8.0  # 1/sqrt(HS)


def _emit(ctx, tc, aps, bpc):
    nc = tc.nc
    xT, wq, wk, wv, wo, tri4, vone, y = aps
    assert bpc % 2 == 0
    nmb = bpc // 2  # macro-batches of 2

    singles = ctx.enter_context(tc.tile_pool(name="singles", bufs=1))
    xpool = ctx.enter_context(tc.tile_pool(name="xp", bufs=6))
    qkpool = ctx.enter_context(tc.tile_pool(name="qkp", bufs=6))
    ppool = ctx.enter_context(tc.tile_pool(name="pp", bufs=6))
    opool = ctx.enter_context(tc.tile_pool(name="op", bufs=12))
    rrpool = ctx.enter_context(tc.tile_pool(name="rrp", bufs=3))
    ypool = ctx.enter_context(tc.tile_pool(name="yp", bufs=4))
    ps_pair = ctx.enter_context(tc.tile_pool(name="ps_pair", bufs=2, space="PSUM"))
    ps_small = ctx.enter_context(tc.tile_pool(name="ps_small", bufs=4, space="PSUM"))

    # Constants / weights, loaded once.
    def _load(name, src, shape, dt=F16):
        t = singles.tile(shape, dt, tag=name, name=name)
        nc.sync.dma_start(out=t, in_=src)
        return t

    wq_sb = [None, None, None]
    wk_sb = [None, None, None]
    wv_sb = [None, None, None]
    wo_sb = [None, None, None]

    def _load_weights():
        # emitted after the first x-load so the Sync queue serves the
        # pipeline-critical transfers in first-use order; q/k weights come
        # in one DMA each (all three 128-row chunks stacked on the free dim)
        wq_big = _load("wqb", wq.rearrange("(c p) d -> p c d", p=128), [128, 3 * D])
        wk_big = _load("wkb", wk.rearrange("(c p) d -> p c d", p=128), [128, 3 * D])
        for i in range(3):
            wq_sb[i] = wq_big.rearrange("p (c d) -> p c d", d=D)[:, i, :]
            wk_sb[i] = wk_big.rearrange("p (c d) -> p c d", d=D)[:, i, :]
        for i in range(3):
            wv_sb[i] = _load(f"wv{i}", wv[i * 128 : (i + 1) * 128, :], [128, D])
        for i in range(3):
            wo_sb[i] = _load(f"wo{i}", wo[i * 128 : (i + 1) * 128, :], [128, D])

    tri4_sb = _load("tri4", tri4, [128, 512])
    tri4v = tri4_sb.rearrange("p (a b c) -> p a b c", b=2, c=128)

    # Persistent v_aug tiles [macro-parity][sub-batch][s-tile]: ones columns
    # are DMA'd once and survive all batches (the per-batch copy writes only
    # cols 0:64 of each 65-wide head block).
    va_all = []

    def _load_va():
        for par in range(2):
            subs = []
            for sub in range(2):
                pair = []
                for st in range(2):
                    t = singles.tile(
                        [128, H * 128],
                        F16,
                        tag=f"va{par}{sub}{st}",
                        name=f"va{par}{sub}{st}",
                    )
                    nc.sync.dma_start(
                        out=t.rearrange("p (h c) -> p h c", c=128)[:, :, 64:128],
                        in_=vone.rearrange("p (h c) -> p h c", c=64),
                    )
                    pair.append(t)
                subs.append(pair)
            va_all.append(subs)

    # xT viewed so two consecutive batches concatenate along the free dim:
    # [mb, d, (sub t)] per 128-row d-chunk
    def x2_src(mb, kc):
        return xT[2 * mb : 2 * mb + 2, kc * 128 : (kc + 1) * 128, :].rearrange(
            "b d t -> d b t"
        )

    # Pipeline state, keyed by global stage index g = mb*6 + sub*3 + hp.
    x_d = {}  # mb -> [3 x-tiles]
    qk_d = {}  # (mb, mt) -> [128, 1024] f16 (q cols 0:512, k cols 512:1024)
    pair_d = {}  # g -> [128, 1024] f32 psum (scores, both heads)
    p_d = {}  # g -> [128, 768] f16 (exp output, both heads)
    o_d = {}  # g -> [128, 512] f32 psum (PV out; rows 64:128 = denominators)
    rr_d = {}  # g -> [128, 512] f32 (rows 64:128 = denominator reciprocals)
    oT_d = {}  # (mb, sub) -> [3 oT tiles]

    def stage(i):
        if 0 <= i < nmb * 6:
            return i // 6, (i % 6) // 3, i % 3  # mb, sub, hp
        return None

    def emit_xload(mb):
        if mb >= nmb:
            return
        eng = nc.scalar if mb == 0 else nc.sync
        tiles = []
        for kc in range(3):
            t = xpool.tile([128, 2 * T], F16, tag="x", name="x")
            eng.dma_start(
                out=t.rearrange("p (b t) -> p b t", t=T), in_=x2_src(mb, kc)
            )
            tiles.append(t)
        x_d[mb] = tiles

    def emit_qkproj(mb, mt):
        x_sb = x_d[mb]
        ps = ps_pair.tile([128, 1024], F32, tag="pair", name="qk_ps")
        for kc in range(3):
            nc.tensor.matmul(
                ps[:, 0:512],
                wq_sb[kc][:, mt * 128 : (mt + 1) * 128],
                x_sb[kc],
                start=(kc == 0),
                stop=(kc == 2),
            )
        for kc in range(3):
            nc.tensor.matmul(
                ps[:, 512:1024],
                wk_sb[kc][:, mt * 128 : (mt + 1) * 128],
                x_sb[kc],
                start=(kc == 0),
                stop=(kc == 2),
            )
        sb = qkpool.tile([128, 1024], F16, tag="qk", name="qk_sb")
        nc.scalar.copy(sb, ps)
        qk_d[(mb, mt)] = sb

    def emit_vproj(mb, sub):
        x_sb = x_d[mb]
        va_mb = va_all[mb % 2]
        for st in range(2):
            ps = ps_small.tile([128, D], F32, tag="small", name="v_ps")
            for kc in range(3):
                nc.tensor.matmul(
                    ps,
                    x_sb[kc][:, sub * T + st * 128 : sub * T + (st + 1) * 128],
                    wv_sb[kc],
                    start=(kc == 0),
                    stop=(kc == 2),
                )
            va3 = va_mb[sub][st].rearrange("p (h c) -> p h c", c=128)
            src3 = ps.rearrange("p (h e) -> p h e", e=64)
            if st == 0:
                nc.scalar.copy(va3[:, :, 0:64], src3)
            else:
                nc.vector.tensor_copy(va3[:, :, 0:64], src3)

    def emit_scores(g):
        mb, sub, hp = stage(g)
        qk = qk_d[(mb, hp)]
        ps = ps_pair.tile([128, 1024], F32, tag="pair", name="s_ps")
        pair_d[g] = ps
        toff = sub * T
        for hh in range(2):
            rows = slice(hh * 64, (hh + 1) * 64)
            base = hh * 512
            # s-chunk 0: full t (N=256); s-chunk 1: only t >= 128 (N=128)
            nc.tensor.matmul(
                ps[:, base : base + 256],
                qk[rows, 512 + toff : 512 + toff + 128],
                qk[rows, toff : toff + 256],
                start=True,
                stop=True,
            )
            nc.tensor.matmul(
                ps[:, base + 256 : base + 384],
                qk[rows, 512 + toff + 128 : 512 + toff + 256],
                qk[rows, toff + 128 : toff + 256],
                start=True,
                stop=True,
            )

    def emit_exp(g):
        p = ppool.tile([128, 768], F16, tag="p", name="p_sb")
        in3 = pair_d.pop(g).rearrange("p (a c) -> p a c", c=512)[:, :, 0:384]
        nc.scalar.activation(
            p.rearrange("p (a c) -> p a c", c=384), in3, EXP, scale=SCALE
        )
        p_d[g] = p

    def emit_pmask(g):
        # zero the causally-masked halves of the two diagonal blocks per head
        # (post-exp, fp16, SBUF): one strided Pool multiply by a 0/1 mask
        p4 = p_d[g].rearrange("p (a b c) -> p a b c", b=3, c=128)[:, :, 0:3:2, :]
        nc.vector.tensor_mul(p4, p4, tri4v)

    def emit_pv(g):
        mb, sub, hp = stage(g)
        va_pair = va_all[mb % 2][sub]
        p3 = p_d.pop(g).rearrange("p (a c) -> p a c", c=384)
        o = ps_small.tile([128, 512], F32, tag="small", name="o_ps")
        o_d[g] = o
        for hh in range(2):
            h = hp * 2 + hh
            o_h = o[:, hh * 256 : (hh + 1) * 256]
            nc.tensor.matmul(
                o_h,
                va_pair[0][:, h * 128 : (h + 1) * 128],
                p3[:, hh, 0:256],
                start=True,
                stop=False,
            )
            nc.tensor.matmul(
                o_h[:, 128:256],
                va_pair[1][:, h * 128 : (h + 1) * 128],
                p3[:, hh, 256:384],
                start=False,
                stop=True,
            )

    def emit_recip(g):
        # reciprocal over the whole O_aug tile: rows 64:128 hold the
        # PE-replicated denominators (extra V_aug ones-columns), so no
        # partition broadcast is needed; rows 0:64 are unused. Base
        # partition stays 0 (the custom DVE op mishandles nonzero bases).
        rr = rrpool.tile([128, 512], F32, tag="rr", name="rr")
        nc.vector.reciprocal_approx_fast(rr, o_d[g])
        rr_d[g] = rr

    def emit_norm(g):
        mb, sub, hp = stage(g)
        rr = rr_d.pop(g)
        o = o_d.pop(g)
        if hp == 0:
            oT_d[(mb, sub)] = [None, None, None]
        oT = opool.tile([128, T], F16, tag="oT", name="oT")
        oT_d[(mb, sub)][hp] = oT
        for hh in range(2):
            nc.vector.tensor_mul(
                oT[hh * 64 : (hh + 1) * 64, :],
                o[0:64, hh * 256 : (hh + 1) * 256],
                rr[64:128, hh * 256 : (hh + 1) * 256],
            )

    def emit_outproj(mb, sub):
        ib = 2 * mb + sub
        oT = oT_d.pop((mb, sub))
        y_sb = ypool.tile([128, 2 * D], F32, tag="ysb", name="y_sb")
        for tt in range(2):
            yp = ps_small.tile([128, D], F32, tag="small", name="y_ps")
            for kc in range(3):
                nc.tensor.matmul(
                    yp,
                    oT[kc][:, tt * 128 : (tt + 1) * 128],
                    wo_sb[kc],
                    start=(kc == 0),
                    stop=(kc == 2),
                )
            nc.scalar.copy(y_sb[:, tt * D : (tt + 1) * D], yp)
        nc.sync.dma_start(
            out=y[ib].rearrange("(a t) d -> t a d", a=2),
            in_=y_sb.rearrange("p (a d) -> p a d", d=D),
        )

    # ---- slot pipeline ----
    emit_xload(0)
    _load_weights()
    _load_va()
    # warm the PE p-state while the prologue DMAs land: a few dummy matmuls
    # on the already-resident mask tile into a scratch PSUM buffer, so the
    # first projection blob starts at full clock instead of mid-ramp
    warm = ps_small.tile([128, 512], F32, tag="small", name="warm_ps")
    for _ in range(8):
        nc.tensor.matmul(warm, tri4_sb[:, 0:128], tri4_sb, start=True, stop=True)
    nstages = nmb * 6
    for i in range(-2, nstages + 5):
        if stage(i - 3) is not None:
            emit_pv(i - 3)
            emit_recip(i - 3)
            emit_norm(i - 3)
        s_cur = stage(i)
        if s_cur is not None:
            emit_scores(i)
            emit_exp(i)
        # projections for macro-batch m are hoisted: q/k chunks two slots
        # ahead of m's first stage, v right at it, so copies never gate PE
        if i % 6 == 4 and stage(i + 2) is not None:
            m = i // 6 + 1
            emit_qkproj(m, 0)
            emit_qkproj(m, 1)
        if i % 6 == 5 and stage(i + 1) is not None:
            emit_qkproj(i // 6 + 1, 2)
        if s_cur is not None and i % 6 == 0:
            mb = i // 6
            emit_vproj(mb, 0)
            emit_xload(mb + 1)
            x_d.pop(mb - 1, None)
        if s_cur is not None and i % 6 == 1:
            emit_vproj(i // 6, 1)
        if s_cur is not None:
            emit_pmask(i)
        if stage(i - 3) is not None and i - 3 == nstages - 1:
            emit_outproj(nmb - 1, 1)
        if stage(i - 4) is not None:
            mb4, sub4, hp4 = stage(i - 4)
            if hp4 == 2 and (mb4, sub4) in oT_d:
                emit_outproj(mb4, sub4)


def build_nc(bpc=BPC):
    nc = bacc.Bacc(
        "TRN2", target_bir_lowering=False, debug=False, enable_asserts=False
    )
    xT = nc.dram_tensor("xT", [bpc, D, T], F16, kind="ExternalInput").ap()
    wq = nc.dram_tensor("wq", [D, D], F16, kind="ExternalInput").ap()
    wk = nc.dram_tensor("wk", [D, D], F16, kind="ExternalInput").ap()
    wv = nc.dram_tensor("wv", [D, D], F16, kind="ExternalInput").ap()
    wo = nc.dram_tensor("wo", [D, D], F16, kind="ExternalInput").ap()
    tri4 = nc.dram_tensor("tri4", [128, 512], F16, kind="ExternalInput").ap()
    vone = nc.dram_tensor("vone", [128, H * 64], F16, kind="ExternalInput").ap()
    y = nc.dram_tensor("y", [bpc, T, D], F32, kind="ExternalOutput").ap()
    with tile.TileContext(nc) as tc:
        with ExitStack() as ctx:
            _emit(ctx, tc, (xT, wq, wk, wv, wo, tri4, vone, y), bpc)
    nc.finalize()
    return nc


_NC_CACHE = {}


def _get_nc(bpc):
    if bpc not in _NC_CACHE:
        _NC_CACHE[bpc] = build_nc(bpc)
    return _NC_CACHE[bpc]


def prep_inputs(x, W_qkv, W_out, b_out):
    x = np.asarray(x, np.float32)
    W_qkv = np.asarray(W_qkv, np.float32)
    nb = x.shape[0]
    xT = np.ascontiguousarray(x.transpose(0, 2, 1)).astype(np.float16)
    Wq = np.ascontiguousarray(
        W_qkv[:, :, 0:64].transpose(1, 0, 2).reshape(D, D)
    ).astype(np.float16)
    Wk = np.ascontiguousarray(
        W_qkv[:, :, 64:128].transpose(1, 0, 2).reshape(D, D)
    ).astype(np.float16)
    Wv = np.ascontiguousarray(
        W_qkv[:, :, 128:192].transpose(1, 0, 2).reshape(D, D)
    ).astype(np.float16)
    Wo = np.ascontiguousarray(np.asarray(W_out, np.float32)).astype(np.float16)
    tri01 = (np.arange(128)[:, None] <= np.arange(128)[None, :]).astype(np.float16)
    tri4 = np.tile(tri01, (1, 4))
    return xT, Wq, Wk, Wv, Wo, tri4, nb


def run(x, W_qkv, W_out, b_out, trace=False, **spmd_kwargs):
    xT, Wq, Wk, Wv, Wo, tri4, nb = prep_inputs(x, W_qkv, W_out, b_out)
    bpc = nb // NCORES
    assert bpc * NCORES == nb
    nc = _get_nc(bpc)
    shards = xT.reshape(NCORES, bpc, D, T)
    in_maps = [
        {
            "xT": shards[i],
            "wq": Wq,
            "wk": Wk,
            "wv": Wv,
            "wo": Wo,
            "tri4": tri4,
            "vone": np.ones((128, H * 64), np.float16),
        }
        for i in range(NCORES)
    ]
    res = run_bass_kernel_spmd(
        nc, in_maps, list(range(NCORES)), trace=trace, **spmd_kwargs
    )
    y = np.concatenate([res.results[i]["y"] for i in range(NCORES)], axis=0)
    b = np.asarray(b_out, np.float32)
    if b.any():
        y = y + b  # bias applied host-side (zeros in practice)
    return y, res


def kernel(x, W_qkv, W_out, b_out):
    y, _ = run(np.asarray(x), np.asarray(W_qkv), np.asarray(W_out), np.asarray(b_out))
    return y


# revision 35
# speedup vs baseline: 1.0472x; 1.0025x over previous
"""Trainium2 Bass kernel: 6-head causal self-attention (nn_MultiHead).

Strategy: pure data-parallel over batch B=256 across 8 NeuronCores
(32 batches/core, no collectives). Per batch, on-chip layout keeps the
contraction dim on SBUF partitions everywhere:

  host:    x [B,T,D] -> xT [B,D,T] fp16;  W_qkv -> Wq/Wk/Wv [D, H*HS] fp16;
           bias is applied host-side (zeros in practice)
  proj:    qT/kT [(h e), t] = W.T @ xT    (PE; two batches fused, N=512;
           q and k of one 128-row chunk share a 2-bank PSUM tile and one
           ACT copy)
  scores:  S^T [s, t] = kT_h^T @ qT_h per head (K=64), both heads of a
           pair in one 2-bank PSUM tile; causal skip: the s-chunk-1
           matmul only covers t >= 128 (N=128)
  softmax: P = exp(S/8), one paired ACT op [128,2,384] -> fp16 SBUF; the
           causal mask is applied POST-exp as a 0/1 multiply on the
           diagonal blocks (DVE, fp16, one strided op per stage)
  PV:      O_aug [128, t] = V_aug^T @ P^T where V_aug carries 64 extra
           all-ones columns, so rows 64:128 of the PSUM result hold the
           softmax denominators PRE-REPLICATED across 64 partitions (the
           extra matmul rows are free: PE cost depends only on N)
  norm:    fast reciprocal over the whole O_aug tile (DVE custom op
           reading PSUM; base partition must be 0), then
           O^T *= rr[64:128] while copying PSUM->SBUF (DVE) -- no
           partition broadcast or gather anywhere
  out:     y [t, d] = O^T.T @ W_out (PE), ACT copy PSUM->SBUF, one DMA
           per batch

A slot pipeline runs the six (sub-batch, head-pair) stages per
macro-batch: slot i emits PV/recip/normalize(i-3), scores+exp(i),
P-mask(i), out-projection(i-4); q/k projections are hoisted two slots
ahead of each macro-batch boundary and v projections split across its
first two slots, so the PE never drains and PSUM stays within the
8-bank budget (2x2-bank score/proj pairs + 4 single banks). Matmul
operands are fp16 (1 cycle/row streaming); accumulation is fp32 PSUM.
"""

import sys

import numpy as np

if "/opt/trn_rl_repo" not in sys.path:
    sys.path.insert(0, "/opt/trn_rl_repo")

from contextlib import ExitStack

import concourse.bass as bass  # noqa: F401
import concourse.tile as tile
from concourse import bacc, mybir
from concourse.bass_utils import run_bass_kernel_spmd

B, T, D, H, HS = 256, 256, 384, 6, 64
NCORES = 8
BPC = B // NCORES  # batches per core
F32 = mybir.dt.float32
F16 = mybir.dt.float16
EXP = mybir.ActivationFunctionType.Exp
SCALE = 1.0 / 8.0  # 1/sqrt(HS)


def _emit(ctx, tc, aps, bpc):
    nc = tc.nc
    xT, wq, wk, wv, wo, tri4, vone, y = aps
    assert bpc % 2 == 0
    nmb = bpc // 2  # macro-batches of 2

    singles = ctx.enter_context(tc.tile_pool(name="singles", bufs=1))
    xpool = ctx.enter_context(tc.tile_pool(name="xp", bufs=6))
    qkpool = ctx.enter_context(tc.tile_pool(name="qkp", bufs=6))
    ppool = ctx.enter_context(tc.tile_pool(name="pp", bufs=6))
    opool = ctx.enter_context(tc.tile_pool(name="op", bufs=12))
    rrpool = ctx.enter_context(tc.tile_pool(name="rrp", bufs=3))
    ypool = ctx.enter_context(tc.tile_pool(name="yp", bufs=4))
    ps_pair = ctx.enter_context(tc.tile_pool(name="ps_pair", bufs=2, space="PSUM"))
    ps_small = ctx.enter_context(tc.tile_pool(name="ps_small", bufs=4, space="PSUM"))

    # Constants / weights, loaded once.
    def _load(name, src, shape, dt=F16):
        t = singles.tile(shape, dt, tag=name, name=name)
        nc.sync.dma_start(out=t, in_=src)
        return t

    wq_sb = [None, None, None]
    wk_sb = [None, None, None]
    wv_sb = [None, None, None]
    wo_sb = [None, None, None]

    def _load_weights():
        # emitted after the first x-load so the Sync queue serves the
        # pipeline-critical transfers in first-use order; q/k weights come
        # in one DMA each (all three 128-row chunks stacked on the free dim)
        wq_big = _load("wqb", wq.rearrange("(c p) d -> p c d", p=128), [128, 3 * D])
        wk_big = _load("wkb", wk.rearrange("(c p) d -> p c d", p=128), [128, 3 * D])
        for i in range(3):
            wq_sb[i] = wq_big.rearrange("p (c d) -> p c d", d=D)[:, i, :]
            wk_sb[i] = wk_big.rearrange("p (c d) -> p c d", d=D)[:, i, :]
        for i in range(3):
            wv_sb[i] = _load(f"wv{i}", wv[i * 128 : (i + 1) * 128, :], [128, D])
        for i in range(3):
            wo_sb[i] = _load(f"wo{i}", wo[i * 128 : (i + 1) * 128, :], [128, D])

    tri4_sb = _load("tri4", tri4, [128, 512])
    tri4v = tri4_sb.rearrange("p (a b c) -> p a b c", b=2, c=128)

    # Persistent v_aug tiles [macro-parity][sub-batch][s-tile]: ones columns
    # are DMA'd once and survive all batches (the per-batch copy writes only
    # cols 0:64 of each 65-wide head block).
    va_all = []

    def _load_va():
        for par in range(2):
            subs = []
            for sub in range(2):
                pair = []
                for st in range(2):
                    t = singles.tile(
                        [128, H * 128],
                        F16,
                        tag=f"va{par}{sub}{st}",
                        name=f"va{par}{sub}{st}",
                    )
                    nc.sync.dma_start(
                        out=t.rearrange("p (h c) -> p h c", c=128)[:, :, 64:128],
                        in_=vone.rearrange("p (h c) -> p h c", c=64),
                    )
                    pair.append(t)
                subs.append(pair)
            va_all.append(subs)

    # xT viewed so two consecutive batches concatenate along the free dim:
    # [mb, d, (sub t)] per 128-row d-chunk
    def x2_src(mb, kc):
        return xT[2 * mb : 2 * mb + 2, kc * 128 : (kc + 1) * 128, :].rearrange(
            "b d t -> d b t"
        )

    # Pipeline state, keyed by global stage index g = mb*6 + sub*3 + hp.
    x_d = {}  # mb -> [3 x-tiles]
    qk_d = {}  # (mb, mt) -> [128, 1024] f16 (q cols 0:512, k cols 512:1024)
    pair_d = {}  # g -> [128, 1024] f32 psum (scores, both heads)
    p_d = {}  # g -> [128, 768] f16 (exp output, both heads)
    o_d = {}  # g -> [128, 512] f32 psum (PV out; rows 64:128 = denominators)
    rr_d = {}  # g -> [128, 512] f32 (rows 64:128 = denominator reciprocals)
    oT_d = {}  # (mb, sub) -> [3 oT tiles]

    def stage(i):
        if 0 <= i < nmb * 6:
            return i // 6, (i % 6) // 3, i % 3  # mb, sub, hp
        return None

    def emit_xload(mb):
        if mb >= nmb:
            return
        eng = nc.scalar if mb == 0 else nc.sync
        tiles = []
        for kc in range(3):
            t = xpool.tile([128, 2 * T], F16, tag="x", name="x")
            eng.dma_start(
                out=t.rearrange("p (b t) -> p b t", t=T), in_=x2_src(mb, kc)
            )
            tiles.append(t)
        x_d[mb] = tiles

    def emit_qkproj(mb, mt):
        x_sb = x_d[mb]
        ps = ps_pair.tile([128, 1024], F32, tag="pair", name="qk_ps")
        for kc in range(3):
            nc.tensor.matmul(
                ps[:, 0:512],
                wq_sb[kc][:, mt * 128 : (mt + 1) * 128],
                x_sb[kc],
                start=(kc == 0),
                stop=(kc == 2),
            )
        for kc in range(3):
            nc.tensor.matmul(
                ps[:, 512:1024],
                wk_sb[kc][:, mt * 128 : (mt + 1) * 128],
                x_sb[kc],
                start=(kc == 0),
                stop=(kc == 2),
            )
        sb = qkpool.tile([128, 1024], F16, tag="qk", name="qk_sb")
        nc.scalar.copy(sb, ps)
        qk_d[(mb, mt)] = sb

    def emit_vproj(mb, sub):
        x_sb = x_d[mb]
        va_mb = va_all[mb % 2]
        for st in range(2):
            ps = ps_small.tile([128, D], F32, tag="small", name="v_ps")
            for kc in range(3):
                nc.tensor.matmul(
                    ps,
                    x_sb[kc][:, sub * T + st * 128 : sub * T + (st + 1) * 128],
                    wv_sb[kc],
                    start=(kc == 0),
                    stop=(kc == 2),
                )
            va3 = va_mb[sub][st].rearrange("p (h c) -> p h c", c=128)
            src3 = ps.rearrange("p (h e) -> p h e", e=64)
            if st == 0:
                nc.scalar.copy(va3[:, :, 0:64], src3)
            else:
                nc.vector.tensor_copy(va3[:, :, 0:64], src3)

    def emit_scores(g):
        mb, sub, hp = stage(g)
        qk = qk_d[(mb, hp)]
        ps = ps_pair.tile([128, 1024], F32, tag="pair", name="s_ps")
        pair_d[g] = ps
        toff = sub * T
        for hh in range(2):
            rows = slice(hh * 64, (hh + 1) * 64)
            base = hh * 512
            # s-chunk 0: full t (N=256); s-chunk 1: only t >= 128 (N=128)
            nc.tensor.matmul(
                ps[:, base : base + 256],
                qk[rows, 512 + toff : 512 + toff + 128],
                qk[rows, toff : toff + 256],
                start=True,
                stop=True,
            )
            nc.tensor.matmul(
                ps[:, base + 256 : base + 384],
                qk[rows, 512 + toff + 128 : 512 + toff + 256],
                qk[rows, toff + 128 : toff + 256],
                start=True,
                stop=True,
            )

    def emit_exp(g):
        p = ppool.tile([128, 768], F16, tag="p", name="p_sb")
        in3 = pair_d.pop(g).rearrange("p (a c) -> p a c", c=512)[:, :, 0:384]
        nc.scalar.activation(
            p.rearrange("p (a c) -> p a c", c=384), in3, EXP, scale=SCALE
        )
        p_d[g] = p

    def emit_pmask(g):
        # zero the causally-masked halves of the two diagonal blocks per head
        # (post-exp, fp16, SBUF): one strided Pool multiply by a 0/1 mask
        p4 = p_d[g].rearrange("p (a b c) -> p a b c", b=3, c=128)[:, :, 0:3:2, :]
        nc.vector.tensor_mul(p4, p4, tri4v)

    def emit_pv(g):
        mb, sub, hp = stage(g)
        va_pair = va_all[mb % 2][sub]
        p3 = p_d.pop(g).rearrange("p (a c) -> p a c", c=384)
        o = ps_small.tile([128, 512], F32, tag="small", name="o_ps")
        o_d[g] = o
        for hh in range(2):
            h = hp * 2 + hh
            o_h = o[:, hh * 256 : (hh + 1) * 256]
            nc.tensor.matmul(
                o_h,
                va_pair[0][:, h * 128 : (h + 1) * 128],
                p3[:, hh, 0:256],
                start=True,
                stop=False,
            )
            nc.tensor.matmul(
                o_h[:, 128:256],
                va_pair[1][:, h * 128 : (h + 1) * 128],
                p3[:, hh, 256:384],
                start=False,
                stop=True,
            )

    def emit_recip(g):
        # reciprocal over the whole O_aug tile: rows 64:128 hold the
        # PE-replicated denominators (extra V_aug ones-columns), so no
        # partition broadcast is needed; rows 0:64 are unused. Base
        # partition stays 0 (the custom DVE op mishandles nonzero bases).
        rr = rrpool.tile([128, 512], F32, tag="rr", name="rr")
        nc.vector.reciprocal_approx_fast(rr, o_d[g])
        rr_d[g] = rr

    def emit_norm(g):
        mb, sub, hp = stage(g)
        rr = rr_d.pop(g)
        o = o_d.pop(g)
        if hp == 0:
            oT_d[(mb, sub)] = [None, None, None]
        oT = opool.tile([128, T], F16, tag="oT", name="oT")
        oT_d[(mb, sub)][hp] = oT
        for hh in range(2):
            nc.vector.tensor_mul(
                oT[hh * 64 : (hh + 1) * 64, :],
                o[0:64, hh * 256 : (hh + 1) * 256],
                rr[64:128, hh * 256 : (hh + 1) * 256],
            )

    def emit_outproj(mb, sub):
        ib = 2 * mb + sub
        oT = oT_d.pop((mb, sub))
        y_sb = ypool.tile([128, 2 * D], F32, tag="ysb", name="y_sb")
        for tt in range(2):
            yp = ps_small.tile([128, D], F32, tag="small", name="y_ps")
            for kc in range(3):
                nc.tensor.matmul(
                    yp,
                    oT[kc][:, tt * 128 : (tt + 1) * 128],
                    wo_sb[kc],
                    start=(kc == 0),
                    stop=(kc == 2),
                )
            nc.scalar.copy(y_sb[:, tt * D : (tt + 1) * D], yp)
        nc.sync.dma_start(
            out=y[ib].rearrange("(a t) d -> t a d", a=2),
            in_=y_sb.rearrange("p (a d) -> p a d", d=D),
        )

    # ---- slot pipeline ----
    emit_xload(0)
    _load_weights()
    _load_va()
    # warm the PE p-state while the prologue DMAs land: a few dummy matmuls
    # on the already-resident mask tile into a scratch PSUM buffer, so the
    # first projection blob starts at full clock instead of mid-ramp
    warm = ps_small.tile([128, 512], F32, tag="small", name="warm_ps")
    for _ in range(18):
        nc.tensor.matmul(warm, tri4_sb[:, 0:128], tri4_sb, start=True, stop=True)
    nstages = nmb * 6
    for i in range(-2, nstages + 5):
        if stage(i - 3) is not None:
            emit_pv(i - 3)
            emit_recip(i - 3)
            emit_norm(i - 3)
        s_cur = stage(i)
        if s_cur is not None:
            emit_scores(i)
            emit_exp(i)
        # projections for macro-batch m are hoisted: q/k chunks two slots
        # ahead of m's first stage, v right at it, so copies never gate PE
        if i % 6 == 4 and stage(i + 2) is not None:
            m = i // 6 + 1
            emit_qkproj(m, 0)
            emit_qkproj(m, 1)
        if i % 6 == 5 and stage(i + 1) is not None:
            emit_qkproj(i // 6 + 1, 2)
        if s_cur is not None and i % 6 == 0:
            mb = i // 6
            emit_vproj(mb, 0)
            emit_xload(mb + 1)
            x_d.pop(mb - 1, None)
        if s_cur is not None and i % 6 == 1:
            emit_vproj(i // 6, 1)
        if s_cur is not None:
            emit_pmask(i)
        if stage(i - 3) is not None and i - 3 == nstages - 1:
            emit_outproj(nmb - 1, 1)
        if stage(i - 4) is not None:
            mb4, sub4, hp4 = stage(i - 4)
            if hp4 == 2 and (mb4, sub4) in oT_d:
                emit_outproj(mb4, sub4)


def build_nc(bpc=BPC):
    nc = bacc.Bacc(
        "TRN2", target_bir_lowering=False, debug=False, enable_asserts=False
    )
    xT = nc.dram_tensor("xT", [bpc, D, T], F16, kind="ExternalInput").ap()
    wq = nc.dram_tensor("wq", [D, D], F16, kind="ExternalInput").ap()
    wk = nc.dram_tensor("wk", [D, D], F16, kind="ExternalInput").ap()
    wv = nc.dram_tensor("wv", [D, D], F16, kind="ExternalInput").ap()
    wo = nc.dram_tensor("wo", [D, D], F16, kind="ExternalInput").ap()
    tri4 = nc.dram_tensor("tri4", [128, 512], F16, kind="ExternalInput").ap()
    vone = nc.dram_tensor("vone", [128, H * 64], F16, kind="ExternalInput").ap()
    y = nc.dram_tensor("y", [bpc, T, D], F32, kind="ExternalOutput").ap()
    with tile.TileContext(nc) as tc:
        with ExitStack() as ctx:
            _emit(ctx, tc, (xT, wq, wk, wv, wo, tri4, vone, y), bpc)
    nc.finalize()
    return nc


_NC_CACHE = {}


def _get_nc(bpc):
    if bpc not in _NC_CACHE:
        _NC_CACHE[bpc] = build_nc(bpc)
    return _NC_CACHE[bpc]


def prep_inputs(x, W_qkv, W_out, b_out):
    x = np.asarray(x, np.float32)
    W_qkv = np.asarray(W_qkv, np.float32)
    nb = x.shape[0]
    xT = np.ascontiguousarray(x.transpose(0, 2, 1)).astype(np.float16)
    Wq = np.ascontiguousarray(
        W_qkv[:, :, 0:64].transpose(1, 0, 2).reshape(D, D)
    ).astype(np.float16)
    Wk = np.ascontiguousarray(
        W_qkv[:, :, 64:128].transpose(1, 0, 2).reshape(D, D)
    ).astype(np.float16)
    Wv = np.ascontiguousarray(
        W_qkv[:, :, 128:192].transpose(1, 0, 2).reshape(D, D)
    ).astype(np.float16)
    Wo = np.ascontiguousarray(np.asarray(W_out, np.float32)).astype(np.float16)
    tri01 = (np.arange(128)[:, None] <= np.arange(128)[None, :]).astype(np.float16)
    tri4 = np.tile(tri01, (1, 4))
    return xT, Wq, Wk, Wv, Wo, tri4, nb


def run(x, W_qkv, W_out, b_out, trace=False, **spmd_kwargs):
    xT, Wq, Wk, Wv, Wo, tri4, nb = prep_inputs(x, W_qkv, W_out, b_out)
    bpc = nb // NCORES
    assert bpc * NCORES == nb
    nc = _get_nc(bpc)
    shards = xT.reshape(NCORES, bpc, D, T)
    in_maps = [
        {
            "xT": shards[i],
            "wq": Wq,
            "wk": Wk,
            "wv": Wv,
            "wo": Wo,
            "tri4": tri4,
            "vone": np.ones((128, H * 64), np.float16),
        }
        for i in range(NCORES)
    ]
    res = run_bass_kernel_spmd(
        nc, in_maps, list(range(NCORES)), trace=trace, **spmd_kwargs
    )
    y = np.concatenate([res.results[i]["y"] for i in range(NCORES)], axis=0)
    b = np.asarray(b_out, np.float32)
    if b.any():
        y = y + b  # bias applied host-side (zeros in practice)
    return y, res


def kernel(x, W_qkv, W_out, b_out):
    y, _ = run(np.asarray(x), np.asarray(W_qkv), np.asarray(W_out), np.asarray(b_out))
    return y
